# revision 1
# baseline (speedup 1.0000x reference)
"""AdaptiveECE on 8 Trainium2 NeuronCores.

Data-parallel over N=1,000,000 rows: each core streams its 125,000-row
shard of logits [N,128] through SBUF exactly once (the memory-bound part,
64MB/core at ~360GB/s/core) and reduces it to two small per-row tensors:

  - mt[r] = max_c x[r,c]           (VectorE segmented reduce_max)
  - s[r]  = sum_c exp(x[r,c])      (split to balance engines: ~5/16 of rows
                                    via ScalarE per-tile Exp+row-accumulate,
                                    the rest via ScalarE big-op Exp followed
                                    by VectorE segmented reduce_sum)

Both per-row tensors stream out per chunk, overlapping later chunks. The
host finishes with O(N) work as the problem's sharding hint prescribes
("finish ECE on one host"): conf = exp(mt)/s, accuracy =
(logits[r, labels[r]] == mt[r]) — exact, since mt is the bit-exact max —
then the global sort of confidences, equal-count bin edges via interp,
searchsorted binning, per-bin (count, conf_sum, acc_sum), and the ECE.

Measured on silicon: ~249.5us in the device's fast clock state, ~297us in
its ~18%-derated state (engine-busy: VectorE ~226us, ScalarE ~226us, DMA
~193us vs a ~178us HBM roofline for the 64MB/core stream). Schedule: tail
chunk first, 2-t-group ramp-in chunk, 8-t-group body chunks, 2/2-t-group
tapered drain chunks; each chunk's input DMA is split in halves.

With PACK_ARGMAX=True, the kernel instead computes the argmax fully
on-device by packing the column index into the low 7 mantissa bits before
the max-reduce ((x & ~0x7F) | (127-c), a fused scalar_tensor_tensor pass);
that variant needs one extra full VectorE pass (~440us vs ~270us measured).

Layout: each partition line holds G=8 consecutive rows (4KB contiguous DMA
runs). Output column (t*G + j), partition p  <->  shard row t*G*128 + p*G + j.
"""

import sys
import types
from contextlib import ExitStack

import numpy as np

import concourse.bass as bass
import concourse.tile as tile
from concourse import bacc, mybir
from concourse.bass_utils import run_bass_kernel_spmd


def _ensure_ntff_hook():
    """bass_utils imports antenv.axon_hooks when tracing is requested; the
    agent image lacks that module. Recreate it (wired to the axon .so) so a
    stray BASS_TRACE=1 in the environment cannot crash the run."""
    try:
        import antenv.axon_hooks  # noqa: F401
        return
    except ImportError:
        pass
    try:
        import antenv
        import trn_agent_boot.trn_boot as tb

        mod = types.ModuleType("antenv.axon_hooks")
        holder = [None]
        mod.set_axon_ntff_profile_hook = lambda h: holder.__setitem__(0, h)
        mod.get_axon_ntff_profile_hook = lambda: holder[0]
        sys.modules["antenv.axon_hooks"] = mod
        antenv.axon_hooks = mod
        try:
            mod.set_axon_ntff_profile_hook(
                tb._ntff_profile_via_ctypes("/opt/axon/libaxon_pjrt.so")
            )
        except Exception:
            pass
    except Exception:
        pass


_ensure_ntff_hook()

N = 1_000_000
C = 128
NBINS = 15
NCORES = 8
ROWS = N // NCORES  # 125_000 per core
MASK = 0xFFFFFF80
G = 8  # rows per partition line (4KB contiguous DMA runs)
JA = 5  # of each 16 columns, this many row-sums go to ScalarE accum
CHUNK_T = 8  # t-groups (of G*128 rows) per chunk
PACK_ARGMAX = False  # False: mt = plain row max; acc via host label-gather

_CACHE: dict = {}
LAST_RESULT = None  # BassKernelResults of the most recent device run


def _build(rows: int, chunk_t: int = CHUNK_T):
    gr = G * 128  # rows per t-group
    tfull = rows // gr  # full t-groups
    tail = rows - tfull * gr  # leftover rows
    tail_p = tail // G  # tail partitions (tail must divide by G)
    assert tail % G == 0, (rows, tail)
    tt = (tfull + (1 if tail else 0)) * G  # output columns

    nc = bacc.Bacc("TRN2", target_bir_lowering=False, debug=False)
    lg = nc.dram_tensor("logits", [rows, C], mybir.dt.float32, kind="ExternalInput").ap()
    s_d = nc.dram_tensor("s", [128, tt], mybir.dt.float32, kind="ExternalOutput").ap()
    mt_d = nc.dram_tensor("mt", [128, tt], mybir.dt.float32, kind="ExternalOutput").ap()

    # [p, t, (j c)] view: row t*1024 + p*8 + j; (j c) is 4KB-contiguous per (p,t)
    lg_t = (
        lg[0 : tfull * gr, :].rearrange("(t p j) c -> p t (j c)", p=128, j=G)
        if tfull
        else None
    )

    with tile.TileContext(nc) as tc, ExitStack() as ctx:
        singles = ctx.enter_context(tc.tile_pool(name="singles", bufs=1))
        xpool = ctx.enter_context(tc.tile_pool(name="x", bufs=3))
        bpool = ctx.enter_context(tc.tile_pool(name="xb", bufs=2))
        epool = ctx.enter_context(tc.tile_pool(name="e", bufs=2))
        spool = ctx.enter_context(tc.tile_pool(name="scratch", bufs=2))

        if PACK_ARGMAX:
            pat = singles.tile([128, C], mybir.dt.uint32)
            nc.gpsimd.iota(pat[:], pattern=[[-1, C]], base=127, channel_multiplier=0)
            maskt = singles.tile([128, 1], mybir.dt.uint32)
            nc.vector.memset(maskt[:], MASK)

        sraw = singles.tile([128, tt], mybir.dt.float32)
        mt_sb = singles.tile([128, tt], mybir.dt.float32)

        # schedule: tail chunk first (its memset off the drain path), a small
        # ramp-in chunk, big chunks, then tapered chunks to shorten the drain
        chunks = []
        t0 = 0
        first = True
        while t0 < tfull:
            left = tfull - t0
            if first:
                n = min(2, left)
                first = False
            elif left > chunk_t + 6:
                n = chunk_t
            elif left > 6:
                n = left - 6
            elif left > 4:
                n = left - 4
            elif left > 2:
                n = left - 2
            else:
                n = left
            chunks.append([t0, n, False])
            t0 += n
        if tail:
            chunks.insert(0, [tfull, 0, True])

        for t0, nfull, has_tail in chunks:
            nt = nfull + (1 if has_tail else 0)
            ncols = nt * G  # output columns this chunk
            x = xpool.tile([128, ncols, C], mybir.dt.float32)
            for h0, h1 in ((0, nfull // 2), (nfull // 2, nfull)):
                if h1 > h0:
                    nc.sync.dma_start(
                        x[:, h0 * G : h1 * G, :].rearrange(
                            "p a c -> p (a c)"
                        ).rearrange("p (t b) -> p t b", b=G * C),
                        lg_t[:, t0 + h0 : t0 + h1, :],
                    )
            if has_tail:
                nc.vector.memset(x[:, nfull * G :, :], 0.0)
                tail_src = lg[tfull * gr : rows, :].rearrange("(p j) c -> p (j c)", j=G)
                nc.sync.dma_start(
                    x[0:tail_p, nfull * G :, :].rearrange("p a c -> p (a c)"), tail_src
                )

            if PACK_ARGMAX:
                xu = x[:].bitcast(mybir.dt.uint32)
                xb = bpool.tile([128, ncols, C], mybir.dt.uint32)
                pat_ap = pat[:]
                pat_bc = bass.AP(
                    tensor=pat_ap.tensor, offset=pat_ap.offset,
                    ap=[list(pat_ap.ap[0]), [0, ncols], list(pat_ap.ap[1])],
                )
                # xb = (x & ~0x7F) | (127-c): one fused DVE pass
                nc.vector.scalar_tensor_tensor(
                    xb[:], xu, maskt[:], pat_bc,
                    op0=mybir.AluOpType.bitwise_and, op1=mybir.AluOpType.bitwise_or,
                )
                nc.vector.reduce_max(
                    mt_sb[:, t0 * G : t0 * G + ncols],
                    xb[:].bitcast(mybir.dt.float32),
                    axis=mybir.AxisListType.X,
                )
            else:
                nc.vector.reduce_max(
                    mt_sb[:, t0 * G : t0 * G + ncols], x[:],
                    axis=mybir.AxisListType.X,
                )
            # row sums of exp(x): within each chunk, the first ka columns go
            # through ScalarE per-tile exp+accum, the rest through one big
            # ScalarE exp + VectorE segmented reduce — balances both engines
            # with only contiguous APs.
            ka = (ncols * JA) // 16
            for col in range(ka):
                scr = spool.tile([128, C], mybir.dt.float32)
                nc.scalar.activation(
                    scr[:], x[:, col, :], mybir.ActivationFunctionType.Exp,
                    accum_out=sraw[:, t0 * G + col : t0 * G + col + 1],
                )
            if ka < ncols:
                e = epool.tile([128, ncols - ka, C], mybir.dt.float32)
                nc.scalar.activation(
                    e[:], x[:, ka:ncols, :], mybir.ActivationFunctionType.Exp
                )
                nc.vector.reduce_sum(
                    sraw[:, t0 * G + ka : t0 * G + ncols], e[:],
                    axis=mybir.AxisListType.X,
                )

            # stream this chunk's outputs out — conf = exp(max)/sum is O(1)
            # per row and finishes on the host with the rest of the ECE math
            lo, hi = t0 * G, t0 * G + ncols
            nc.sync.dma_start(s_d[:, lo:hi], sraw[:, lo:hi])
            nc.sync.dma_start(mt_d[:, lo:hi], mt_sb[:, lo:hi])

    nc.compile()
    return nc


def _unpermute(a_2d, rows):
    """Device output [128, TT] -> per-row vector [rows].

    Column t*G+j, partition p <-> row t*G*128 + p*G + j.
    """
    gr = G * 128
    tfull = rows // gr
    tail = rows - tfull * gr
    tail_p = tail // G
    out = np.empty(rows, a_2d.dtype)
    nmain = tfull * gr
    out[:nmain] = (
        a_2d[:, : tfull * G].reshape(128, tfull, G).transpose(1, 0, 2).reshape(-1)
    )
    if tail:
        out[nmain:] = a_2d[:tail_p, tfull * G :].reshape(-1)
    return out


def _finish(conf, acc):
    """Mirror of the reference ECE finishing on host."""
    n = conf.shape[0]
    sorted_conf = np.sort(conf)
    q = np.linspace(0.0, float(n), NBINS + 1, dtype=np.float32)
    edges = np.interp(q, np.arange(n, dtype=np.float32), sorted_conf).astype(np.float32)
    idx = np.searchsorted(edges[1:-1], conf, side="left")
    valid = (conf > edges[0]) & (conf <= edges[-1])
    idx = np.where(valid, idx, NBINS)
    cnt = np.bincount(idx, minlength=NBINS + 1)[:NBINS].astype(np.float32)
    csum = np.bincount(idx, weights=conf.astype(np.float64), minlength=NBINS + 1)[
        :NBINS
    ].astype(np.float32)
    asum = np.bincount(idx, weights=acc.astype(np.float64), minlength=NBINS + 1)[
        :NBINS
    ].astype(np.float32)
    prop = cnt / np.float32(n)
    safe = np.maximum(cnt, 1.0)
    gap = np.abs(csum / safe - asum / safe)
    ece = np.sum(np.where(cnt > 0, gap * prop, 0.0), dtype=np.float32)
    return np.asarray(ece, dtype=np.float32).reshape(1)


def kernel(logits, labels, trace: bool = False):
    global LAST_RESULT
    logits = np.asarray(logits)
    labels = np.asarray(labels)
    assert logits.shape == (N, C), logits.shape

    if "nc" not in _CACHE:
        _CACHE["nc"] = _build(ROWS)
    nc = _CACHE["nc"]

    in_maps = [
        {"logits": np.ascontiguousarray(logits[i * ROWS : (i + 1) * ROWS], np.float32)}
        for i in range(NCORES)
    ]
    res = run_bass_kernel_spmd(nc, in_maps, core_ids=list(range(NCORES)), trace=trace)
    LAST_RESULT = res

    s = np.empty(N, np.float32)
    mt = np.empty(N, np.float32)
    for i in range(NCORES):
        s[i * ROWS : (i + 1) * ROWS] = _unpermute(res.results[i]["s"], ROWS)
        mt[i * ROWS : (i + 1) * ROWS] = _unpermute(res.results[i]["mt"], ROWS)

    if PACK_ARGMAX:
        pred = 127 - (mt.view(np.uint32) & np.uint32(0x7F))
        acc = (pred.astype(np.int64) == labels.astype(np.int64)).astype(np.float32)
        m = (mt.view(np.uint32) & np.uint32(MASK)).view(np.float32)
    else:
        # mt = exact per-row max (f32); accuracy = logit at the label equals it
        xlab = logits[np.arange(N), labels.astype(np.int64)]
        acc = (xlab == mt).astype(np.float32)
        m = mt
    conf = (np.exp(m) / s).astype(np.float32)
    return _finish(conf, acc)



# revision 3
# speedup vs baseline: 1.1236x; 1.1236x over previous
"""AdaptiveECE on 8 Trainium2 NeuronCores — v2 (PE-offloaded softmax sums).

Data-parallel over N=1,000,000 rows: each core streams its 125,000-row shard
of logits [N,128] through SBUF once (64MB/core, ~179us at the 358GB/s/core
HBM roofline) and reduces it to two per-row scalars:

  - mt[r] = max_c x[r,c]       exact f32 (VectorE segmented reduce_max, the
                               only 1x-rate DVE pass we keep: ~131us)
  - s[r]  = sum_c exp(x[r,c])  via TensorE+ScalarE instead of DVE:
      1. PE transposes each [128 rows, 128 cols] f32 block into PSUM
         (is_transpose matmul vs identity, 2 cyc/row: ~105us)
      2. ScalarE computes exp on the PSUM-resident transposed block, writing
         bf16 to SBUF (1 elem/cycle/lane @1.2GHz, dtype-free: ~123us)
      3. PE contracts the transposed exp over partitions (=columns) with a
         sliding one-hot stationary so each 512-row block's sums land on its
         own PSUM partition; 128 blocks accumulate into one PSUM bank
         (bf16 matmul 1 cyc/row: ~53us), then one cheap DVE copy evacuates
         65,536 row-sums at once.

  v1 ran both segmented reductions on DVE at its 1x tensor_reduce rate plus
  many small ScalarE accum ops (~226us busy on each), so compute — not the
  64MB stream — was the critical path (249us). v2 puts every engine below
  the DMA roofline: DVE ~135us, ACT ~125us, PE ~160us, DMA ~181us.

The host finishes with O(N) work as the sharding hint prescribes ("finish
ECE on one host"): conf = exp(mt)/s, accuracy = (logits[r, labels[r]] ==
mt[r]) — exact since mt is the bit-exact max — then the global sort,
equal-count bin edges, per-bin (count, conf_sum, acc_sum), and the ECE.

Layout: each partition line holds G=8 consecutive rows (4KB contiguous DMA
runs). mt column (t*G + j), partition p  <->  shard row t*G*128 + p*G + j.
Sums come out block-indexed: s_d[k, S*512 + m] = sum of row g*1024 + p*8 +
(4h + m//128) with p = m%128, where B = S*128 + k = 2g + h.
"""

import sys
import types
from contextlib import ExitStack

import numpy as np

import concourse.bass as bass
import concourse.tile as tile
from concourse import bacc, mybir
from concourse.bass_utils import run_bass_kernel_spmd
from concourse.masks import make_identity


def _ensure_ntff_hook():
    """bass_utils imports antenv.axon_hooks when tracing is requested; the
    agent image lacks that module. Recreate it (wired to the axon .so) so a
    stray BASS_TRACE=1 in the environment cannot crash the run."""
    try:
        import antenv.axon_hooks  # noqa: F401
        return
    except ImportError:
        pass
    try:
        import antenv
        import trn_agent_boot.trn_boot as tb

        mod = types.ModuleType("antenv.axon_hooks")
        holder = [None]
        mod.set_axon_ntff_profile_hook = lambda h: holder.__setitem__(0, h)
        mod.get_axon_ntff_profile_hook = lambda: holder[0]
        sys.modules["antenv.axon_hooks"] = mod
        antenv.axon_hooks = mod
        try:
            mod.set_axon_ntff_profile_hook(
                tb._ntff_profile_via_ctypes("/opt/axon/libaxon_pjrt.so")
            )
        except Exception:
            pass
    except Exception:
        pass


_ensure_ntff_hook()

N = 1_000_000
C = 128
NBINS = 15
NCORES = 8
ROWS = N // NCORES  # 125_000 per core
G = 8  # rows per partition line (4KB contiguous DMA runs)
GR = G * 128  # rows per t-group
TFULL = ROWS // GR  # 122 full t-groups
TAIL = ROWS - TFULL * GR  # 72 leftover rows
TAIL_P = TAIL // G  # 9 tail partitions
NT = TFULL + 1  # t-groups incl. zero-padded tail
NBLK = 2 * NT  # 512-row sum blocks
NSG = (NBLK + 127) // 128  # PSUM sum groups (2)
CHUNK_T = 8  # t-groups per DMA chunk
M_DELAY = 2  # t-groups between exp and its sum-matmuls (keeps PE unstalled)

_CACHE: dict = {}
LAST_RESULT = None  # BassKernelResults of the most recent device run


def _build(rows: int, chunk_t: int = CHUNK_T):
    tfull = rows // GR
    tail = rows - tfull * GR
    tail_p = tail // G
    assert tail % G == 0, (rows, tail)
    nt = tfull + (1 if tail else 0)
    tt = nt * G  # mt output columns
    nblk = 2 * nt
    nsg = (nblk + 127) // 128

    nc = bacc.Bacc("TRN2", target_bir_lowering=False, debug=False)
    lg = nc.dram_tensor("logits", [rows, C], mybir.dt.float32, kind="ExternalInput").ap()
    s_d = nc.dram_tensor("s", [128, nsg * 512], mybir.dt.float32, kind="ExternalOutput").ap()
    mt_d = nc.dram_tensor("mt", [128, tt], mybir.dt.float32, kind="ExternalOutput").ap()

    # [p, t, (j c)] view: row t*1024 + p*8 + j; (j c) is 4KB-contiguous per (p,t)
    lg_t = (
        lg[0 : tfull * GR, :].rearrange("(t p j) c -> p t (j c)", p=128, j=G)
        if tfull
        else None
    )

    with tile.TileContext(nc) as tc, ExitStack() as ctx:
        singles = ctx.enter_context(tc.tile_pool(name="singles", bufs=1))
        xpool = ctx.enter_context(tc.tile_pool(name="x", bufs=3))
        epool = ctx.enter_context(tc.tile_pool(name="e", bufs=2 + M_DELAY))
        tpsum = ctx.enter_context(tc.tile_pool(name="tp", bufs=2, space="PSUM"))
        spsum = ctx.enter_context(tc.tile_pool(name="sp", bufs=nsg, space="PSUM"))

        ident = singles.tile([128, 128], mybir.dt.float32)
        make_identity(nc, ident[:])
        # sliding one-hot stationary: onehot[:, 127-k : 255-k] has its 1 at col k
        onehot = singles.tile([128, 255], mybir.dt.bfloat16)
        nc.vector.memset(onehot[:], 0.0)
        nc.vector.memset(onehot[:, 127:128], 1.0)

        mt_sb = singles.tile([128, tt], mybir.dt.float32)
        s_sb = singles.tile([128, nsg * 512], mybir.dt.float32)
        s_ps = [
            spsum.tile([128, 512], mybir.dt.float32, name=f"s_ps{i}")
            for i in range(nsg)
        ]
        s_count = [0] * nsg  # matmuls issued into each sum group
        s_total = [0] * nsg  # matmuls each group will receive
        for b in range(nblk):
            s_total[b // 128] += 1

        # chunk schedule: tail first (its memset off the drain path), 2-t-group
        # ramp-in, 8-t-group body, 2/2/2-t-group taper to shorten the drain
        chunks = []
        t0 = 0
        first = True
        while t0 < tfull:
            left = tfull - t0
            if first:
                n = min(2, left)
                first = False
            elif left > chunk_t + 6:
                n = chunk_t
            elif left > 6:
                n = left - 6
            elif left > 4:
                n = left - 4
            elif left > 2:
                n = left - 2
            else:
                n = left
            chunks.append([t0, n, False])
            t0 += n
        if tail:
            chunks.insert(0, [tfull, 0, True])

        pending_m = []  # (et_tile, global_t) awaiting their sum-matmuls

        def flush_m(limit):
            while len(pending_m) > limit:
                et, gt = pending_m.pop(0)
                for h in (0, 1):
                    b = 2 * gt + h
                    sg = b // 128
                    k = b % 128
                    nc.tensor.matmul(
                        s_ps[sg][:],
                        onehot[:, 127 - k : 255 - k],
                        et[:, h * 512 : (h + 1) * 512],
                        start=(s_count[sg] == 0),
                        stop=(s_count[sg] == s_total[sg] - 1),
                        skip_group_check=True,
                    )
                    s_count[sg] += 1

        for t0, nfull, has_tail in chunks:
            ntg = nfull + (1 if has_tail else 0)
            ncols = ntg * G
            x = xpool.tile([128, ncols, C], mybir.dt.float32)
            for h0, h1 in ((0, nfull // 2), (nfull // 2, nfull)):
                if h1 > h0:
                    nc.sync.dma_start(
                        x[:, h0 * G : h1 * G, :].rearrange(
                            "p a c -> p (a c)"
                        ).rearrange("p (t b) -> p t b", b=G * C),
                        lg_t[:, t0 + h0 : t0 + h1, :],
                    )
            if has_tail:
                nc.vector.memset(x[:, nfull * G :, :], 0.0)
                tail_src = lg[tfull * GR : rows, :].rearrange("(p j) c -> p (j c)", j=G)
                nc.sync.dma_start(
                    x[0:tail_p, nfull * G :, :].rearrange("p a c -> p (a c)"), tail_src
                )

            # exact row max on DVE (the one 1x pass we keep)
            nc.vector.reduce_max(
                mt_sb[:, t0 * G : t0 * G + ncols], x[:],
                axis=mybir.AxisListType.X,
            )

            for lt in range(ntg):
                gt = t0 + lt  # global t-group id
                tp = tpsum.tile([128, 1024], mybir.dt.float32)
                for j in range(8):
                    nc.tensor.matmul(
                        tp[:, j * 128 : (j + 1) * 128],
                        x[:, lt * G + j, :],
                        ident[:],
                        is_transpose=True,
                        skip_group_check=True,
                    )
                et = epool.tile([128, 1024], mybir.dt.bfloat16)
                nc.scalar.activation(
                    et[:], tp[:], mybir.ActivationFunctionType.Exp
                )
                pending_m.append((et, gt))
                flush_m(M_DELAY)

            # stream this chunk's maxes out
            lo, hi = t0 * G, t0 * G + ncols
            nc.sync.dma_start(mt_d[:, lo:hi], mt_sb[:, lo:hi])

        flush_m(0)
        for sg in range(nsg):
            nc.vector.tensor_copy(
                s_sb[:, sg * 512 : (sg + 1) * 512], s_ps[sg][:]
            )
        nc.sync.dma_start(s_d[:], s_sb[:])

    nc.compile()
    return nc


def _unpermute_mt(a_2d, rows):
    """Device mt [128, TT] -> per-row vector [rows].

    Column t*G+j, partition p <-> row t*G*128 + p*G + j.
    """
    tfull = rows // GR
    tail = rows - tfull * GR
    tail_p = tail // G
    out = np.empty(rows, a_2d.dtype)
    nmain = tfull * GR
    out[:nmain] = (
        a_2d[:, : tfull * G].reshape(128, tfull, G).transpose(1, 0, 2).reshape(-1)
    )
    if tail:
        out[nmain:] = a_2d[:tail_p, tfull * G :].reshape(-1)
    return out


def _unpermute_s(s_2d, rows):
    """Device s [128, NSG*512] -> per-row sum vector [rows].

    s_2d[k, S*512 + m] = sum for block B = S*128 + k, which covers row
    g*1024 + p*8 + j with g = B//2, h = B%2, j = 4h + m//128, p = m%128.
    """
    tfull = rows // GR
    tail = rows - tfull * GR
    nt = tfull + (1 if tail else 0)
    nblk = 2 * nt
    nsg = (nblk + 127) // 128
    blocks = (
        s_2d.reshape(128, nsg, 512).transpose(1, 0, 2).reshape(nsg * 128, 512)[:nblk]
    )
    # [B, m] -> [g, h, j', p] -> row-major (g, p, j=(h,j'))
    s_rows = blocks.reshape(nt, 2, 4, 128).transpose(0, 3, 1, 2)
    return s_rows.reshape(-1)[:rows].copy()


def _finish(conf, acc):
    """Mirror of the reference ECE finishing on host."""
    n = conf.shape[0]
    sorted_conf = np.sort(conf)
    q = np.linspace(0.0, float(n), NBINS + 1, dtype=np.float32)
    edges = np.interp(q, np.arange(n, dtype=np.float32), sorted_conf).astype(np.float32)
    idx = np.searchsorted(edges[1:-1], conf, side="left")
    valid = (conf > edges[0]) & (conf <= edges[-1])
    idx = np.where(valid, idx, NBINS)
    cnt = np.bincount(idx, minlength=NBINS + 1)[:NBINS].astype(np.float32)
    csum = np.bincount(idx, weights=conf.astype(np.float64), minlength=NBINS + 1)[
        :NBINS
    ].astype(np.float32)
    asum = np.bincount(idx, weights=acc.astype(np.float64), minlength=NBINS + 1)[
        :NBINS
    ].astype(np.float32)
    prop = cnt / np.float32(n)
    safe = np.maximum(cnt, 1.0)
    gap = np.abs(csum / safe - asum / safe)
    ece = np.sum(np.where(cnt > 0, gap * prop, 0.0), dtype=np.float32)
    return np.asarray(ece, dtype=np.float32).reshape(1)


def kernel(logits, labels, trace: bool = False):
    global LAST_RESULT
    logits = np.asarray(logits)
    labels = np.asarray(labels)
    assert logits.shape == (N, C), logits.shape

    if "nc" not in _CACHE:
        _CACHE["nc"] = _build(ROWS)
    nc = _CACHE["nc"]

    in_maps = [
        {"logits": np.ascontiguousarray(logits[i * ROWS : (i + 1) * ROWS], np.float32)}
        for i in range(NCORES)
    ]
    res = run_bass_kernel_spmd(nc, in_maps, core_ids=list(range(NCORES)), trace=trace)
    LAST_RESULT = res

    s = np.empty(N, np.float32)
    mt = np.empty(N, np.float32)
    for i in range(NCORES):
        s[i * ROWS : (i + 1) * ROWS] = _unpermute_s(res.results[i]["s"], ROWS)
        mt[i * ROWS : (i + 1) * ROWS] = _unpermute_mt(res.results[i]["mt"], ROWS)

    # mt = exact per-row max (f32); accuracy = logit at the label equals it
    xlab = logits[np.arange(N), labels.astype(np.int64)]
    acc = (xlab == mt).astype(np.float32)
    conf = (np.exp(mt) / s).astype(np.float32)
    return _finish(conf, acc)


# revision 8
# speedup vs baseline: 1.1389x; 1.0136x over previous
"""AdaptiveECE on 8 Trainium2 NeuronCores — v2 (PE-offloaded softmax sums).

Data-parallel over N=1,000,000 rows: each core streams its 125,000-row shard
of logits [N,128] through SBUF once (64MB/core, ~179us at the 358GB/s/core
HBM roofline) and reduces it to two per-row scalars:

  - mt[r] = max_c x[r,c]       exact f32 (VectorE segmented reduce_max, the
                               only 1x-rate DVE pass we keep: ~131us)
  - s[r]  = sum_c exp(x[r,c])  via TensorE+ScalarE instead of DVE:
      1. PE transposes each [128 rows, 128 cols] f32 block into PSUM
         (is_transpose matmul vs identity, 2 cyc/row: ~105us)
      2. ScalarE computes exp on the PSUM-resident transposed block, writing
         bf16 to SBUF (1 elem/cycle/lane @1.2GHz, dtype-free: ~123us)
      3. PE contracts the transposed exp over partitions (=columns) with a
         sliding one-hot stationary so each 512-row block's sums land on its
         own PSUM partition; 128 blocks accumulate into one PSUM bank
         (bf16 matmul 1 cyc/row: ~53us), then one cheap DVE copy evacuates
         65,536 row-sums at once.

  v1 ran both segmented reductions on DVE at its 1x tensor_reduce rate plus
  many small ScalarE accum ops (~226us busy on each), so compute — not the
  64MB stream — was the critical path (249us). v2 puts every engine below
  the DMA roofline: DVE ~135us, ACT ~125us, PE ~160us, DMA ~181us.

The host finishes with O(N) work as the sharding hint prescribes ("finish
ECE on one host"): conf = exp(mt)/s, accuracy = (logits[r, labels[r]] ==
mt[r]) — exact since mt is the bit-exact max — then the global sort,
equal-count bin edges, per-bin (count, conf_sum, acc_sum), and the ECE.

Layout: each partition line holds G=8 consecutive rows (4KB contiguous DMA
runs). mt column (t*G + j), partition p  <->  shard row t*G*128 + p*G + j.
Sums come out block-indexed: s_d[k, S*512 + m] = sum of row g*1024 + p*8 +
(4h + m//128) with p = m%128, where B = S*128 + k = 2g + h.
"""

import sys
import types
from contextlib import ExitStack

import numpy as np

import concourse.bass as bass
import concourse.tile as tile
from concourse import bacc, mybir
from concourse.bass_utils import run_bass_kernel_spmd
from concourse.masks import make_identity


def _ensure_ntff_hook():
    """bass_utils imports antenv.axon_hooks when tracing is requested; the
    agent image lacks that module. Recreate it (wired to the axon .so) so a
    stray BASS_TRACE=1 in the environment cannot crash the run."""
    try:
        import antenv.axon_hooks  # noqa: F401
        return
    except ImportError:
        pass
    try:
        import antenv
        import trn_agent_boot.trn_boot as tb

        mod = types.ModuleType("antenv.axon_hooks")
        holder = [None]
        mod.set_axon_ntff_profile_hook = lambda h: holder.__setitem__(0, h)
        mod.get_axon_ntff_profile_hook = lambda: holder[0]
        sys.modules["antenv.axon_hooks"] = mod
        antenv.axon_hooks = mod
        try:
            mod.set_axon_ntff_profile_hook(
                tb._ntff_profile_via_ctypes("/opt/axon/libaxon_pjrt.so")
            )
        except Exception:
            pass
    except Exception:
        pass


_ensure_ntff_hook()

N = 1_000_000
C = 128
NBINS = 15
NCORES = 8
ROWS = N // NCORES  # 125_000 per core
G = 8  # rows per partition line (4KB contiguous DMA runs)
GR = G * 128  # rows per t-group
TFULL = ROWS // GR  # 122 full t-groups
TAIL = ROWS - TFULL * GR  # 72 leftover rows
TAIL_P = TAIL // G  # 9 tail partitions
NT = TFULL + 1  # t-groups incl. zero-padded tail
NBLK = 2 * NT  # 512-row sum blocks
NSG = (NBLK + 127) // 128  # PSUM sum groups (2)
CHUNK_T = 8  # t-groups per DMA chunk
M_DELAY = 2  # t-groups between exp and its sum-matmuls (keeps PE unstalled)
USE_FP32R = False  # fp32r transposes: 1.5 cyc/row vs fp32's 2 on the PE

_CACHE: dict = {}
LAST_RESULT = None  # BassKernelResults of the most recent device run


def _build(rows: int, chunk_t: int = CHUNK_T):
    tfull = rows // GR
    tail = rows - tfull * GR
    tail_p = tail // G
    assert tail % G == 0, (rows, tail)
    nt = tfull + (1 if tail else 0)
    tt = nt * G  # mt output columns
    nblk = 2 * nt
    nsg = (nblk + 127) // 128

    nc = bacc.Bacc("TRN2", target_bir_lowering=False, debug=False)
    lg = nc.dram_tensor("logits", [rows, C], mybir.dt.float32, kind="ExternalInput").ap()
    s_d = nc.dram_tensor("s", [128, nsg * 512], mybir.dt.float32, kind="ExternalOutput").ap()
    mt_d = nc.dram_tensor("mt", [128, tt], mybir.dt.float32, kind="ExternalOutput").ap()

    # [p, t, (j c)] view: row t*1024 + p*8 + j; (j c) is 4KB-contiguous per (p,t)
    lg_t = (
        lg[0 : tfull * GR, :].rearrange("(t p j) c -> p t (j c)", p=128, j=G)
        if tfull
        else None
    )

    with tile.TileContext(nc) as tc, ExitStack() as ctx:
        singles = ctx.enter_context(tc.tile_pool(name="singles", bufs=1))
        xpool = ctx.enter_context(tc.tile_pool(name="x", bufs=4))
        epool = ctx.enter_context(tc.tile_pool(name="e", bufs=2 + M_DELAY))
        tpsum = ctx.enter_context(tc.tile_pool(name="tp", bufs=2, space="PSUM"))
        spsum = ctx.enter_context(tc.tile_pool(name="sp", bufs=nsg, space="PSUM"))

        ident = singles.tile([128, 128], mybir.dt.float32)
        make_identity(nc, ident[:])
        ident_bf = singles.tile([128, 128], mybir.dt.bfloat16)
        make_identity(nc, ident_bf[:])
        # sliding one-hot stationary: onehot[:, 127-k : 255-k] has its 1 at col k
        onehot = singles.tile([128, 255], mybir.dt.bfloat16)
        nc.vector.memset(onehot[:], 0.0)
        nc.vector.memset(onehot[:, 127:128], 1.0)

        mt_sb = singles.tile([128, tt], mybir.dt.float32)
        s_sb = singles.tile([128, nsg * 512], mybir.dt.float32)
        s_ps = [
            spsum.tile([128, 512], mybir.dt.float32, name=f"s_ps{i}")
            for i in range(nsg)
        ]
        s_count = [0] * nsg  # matmuls issued into each sum group
        s_total = [0] * nsg  # matmuls each group will receive
        for b in range(nblk):
            s_total[b // 128] += 1

        # chunk schedule: tail first (its memset off the drain path), 2-t-group
        # ramp-in, 8-t-group body, 2/2/2-t-group taper to shorten the drain
        chunks = []
        t0 = 0
        ramp = [1, 2]
        while t0 < tfull:
            left = tfull - t0
            if ramp:
                n = min(ramp.pop(0), left)
            elif left > chunk_t + 6:
                n = chunk_t
            elif left > 6:
                n = left - 6
            elif left > 4:
                n = left - 4
            elif left > 2:
                n = left - 2
            else:
                n = left
            chunks.append([t0, n, False])
            t0 += n
        if tail:
            chunks.insert(1, [tfull, 0, True])

        pending_m = []  # (et_tile, global_t) awaiting their sum-matmuls

        def flush_m(limit):
            while len(pending_m) > limit:
                et, gt = pending_m.pop(0)
                for h in (0, 1):
                    b = 2 * gt + h
                    sg = b // 128
                    k = b % 128
                    nc.tensor.matmul(
                        s_ps[sg][:],
                        onehot[:, 127 - k : 255 - k],
                        et[:, h * 512 : (h + 1) * 512],
                        start=(s_count[sg] == 0),
                        stop=(s_count[sg] == s_total[sg] - 1),
                        skip_group_check=True,
                    )
                    s_count[sg] += 1

        for t0, nfull, has_tail in chunks:
            ntg = nfull + (1 if has_tail else 0)
            ncols = ntg * G
            x = xpool.tile([128, ncols, C], mybir.dt.float32)
            for h0, h1 in ((0, nfull // 2), (nfull // 2, nfull)):
                if h1 > h0:
                    nc.sync.dma_start(
                        x[:, h0 * G : h1 * G, :].rearrange(
                            "p a c -> p (a c)"
                        ).rearrange("p (t b) -> p t b", b=G * C),
                        lg_t[:, t0 + h0 : t0 + h1, :],
                    )
            if has_tail:
                nc.vector.memset(x[:, nfull * G :, :], 0.0)
                tail_src = lg[tfull * GR : rows, :].rearrange("(p j) c -> p (j c)", j=G)
                nc.sync.dma_start(
                    x[0:tail_p, nfull * G :, :].rearrange("p a c -> p (a c)"), tail_src
                )

            # exact row max on DVE (the one 1x pass we keep)
            nc.vector.reduce_max(
                mt_sb[:, t0 * G : t0 * G + ncols], x[:],
                axis=mybir.AxisListType.X,
            )

            for lt in range(ntg):
                gt = t0 + lt  # global t-group id
                tp = tpsum.tile([128, 1024], mybir.dt.float32)
                for j in range(8):
                    if USE_FP32R:
                        nc.tensor.matmul(
                            tp[:, j * 128 : (j + 1) * 128].bitcast(mybir.dt.float32r),
                            x[:, lt * G + j, :].bitcast(mybir.dt.float32r),
                            ident_bf[:],
                            is_transpose=True,
                            skip_group_check=True,
                        )
                    else:
                        nc.tensor.matmul(
                            tp[:, j * 128 : (j + 1) * 128],
                            x[:, lt * G + j, :],
                            ident[:],
                            is_transpose=True,
                            skip_group_check=True,
                        )
                et = epool.tile([128, 1024], mybir.dt.bfloat16)
                nc.scalar.activation(
                    et[:], tp[:], mybir.ActivationFunctionType.Exp
                )
                pending_m.append((et, gt))
                flush_m(M_DELAY)

            # stream this chunk's maxes out
            lo, hi = t0 * G, t0 * G + ncols
            nc.sync.dma_start(mt_d[:, lo:hi], mt_sb[:, lo:hi])

        flush_m(0)
        for sg in range(nsg):
            nc.vector.tensor_copy(
                s_sb[:, sg * 512 : (sg + 1) * 512], s_ps[sg][:]
            )
        nc.sync.dma_start(s_d[:], s_sb[:])

    nc.compile()
    return nc


def _unpermute_mt(a_2d, rows):
    """Device mt [128, TT] -> per-row vector [rows].

    Column t*G+j, partition p <-> row t*G*128 + p*G + j.
    """
    tfull = rows // GR
    tail = rows - tfull * GR
    tail_p = tail // G
    out = np.empty(rows, a_2d.dtype)
    nmain = tfull * GR
    out[:nmain] = (
        a_2d[:, : tfull * G].reshape(128, tfull, G).transpose(1, 0, 2).reshape(-1)
    )
    if tail:
        out[nmain:] = a_2d[:tail_p, tfull * G :].reshape(-1)
    return out


def _unpermute_s(s_2d, rows):
    """Device s [128, NSG*512] -> per-row sum vector [rows].

    s_2d[k, S*512 + m] = sum for block B = S*128 + k, which covers row
    g*1024 + p*8 + j with g = B//2, h = B%2, j = 4h + m//128, p = m%128.
    """
    tfull = rows // GR
    tail = rows - tfull * GR
    nt = tfull + (1 if tail else 0)
    nblk = 2 * nt
    nsg = (nblk + 127) // 128
    blocks = (
        s_2d.reshape(128, nsg, 512).transpose(1, 0, 2).reshape(nsg * 128, 512)[:nblk]
    )
    # [B, m] -> [g, h, j', p] -> row-major (g, p, j=(h,j'))
    s_rows = blocks.reshape(nt, 2, 4, 128).transpose(0, 3, 1, 2)
    return s_rows.reshape(-1)[:rows].copy()


def _finish(conf, acc):
    """Mirror of the reference ECE finishing on host."""
    n = conf.shape[0]
    sorted_conf = np.sort(conf)
    q = np.linspace(0.0, float(n), NBINS + 1, dtype=np.float32)
    edges = np.interp(q, np.arange(n, dtype=np.float32), sorted_conf).astype(np.float32)
    idx = np.searchsorted(edges[1:-1], conf, side="left")
    valid = (conf > edges[0]) & (conf <= edges[-1])
    idx = np.where(valid, idx, NBINS)
    cnt = np.bincount(idx, minlength=NBINS + 1)[:NBINS].astype(np.float32)
    csum = np.bincount(idx, weights=conf.astype(np.float64), minlength=NBINS + 1)[
        :NBINS
    ].astype(np.float32)
    asum = np.bincount(idx, weights=acc.astype(np.float64), minlength=NBINS + 1)[
        :NBINS
    ].astype(np.float32)
    prop = cnt / np.float32(n)
    safe = np.maximum(cnt, 1.0)
    gap = np.abs(csum / safe - asum / safe)
    ece = np.sum(np.where(cnt > 0, gap * prop, 0.0), dtype=np.float32)
    return np.asarray(ece, dtype=np.float32).reshape(1)


def kernel(logits, labels, trace: bool = False):
    global LAST_RESULT
    logits = np.asarray(logits)
    labels = np.asarray(labels)
    assert logits.shape == (N, C), logits.shape

    if "nc" not in _CACHE:
        _CACHE["nc"] = _build(ROWS)
    nc = _CACHE["nc"]

    in_maps = [
        {"logits": np.ascontiguousarray(logits[i * ROWS : (i + 1) * ROWS], np.float32)}
        for i in range(NCORES)
    ]
    res = run_bass_kernel_spmd(nc, in_maps, core_ids=list(range(NCORES)), trace=trace)
    LAST_RESULT = res

    s = np.empty(N, np.float32)
    mt = np.empty(N, np.float32)
    for i in range(NCORES):
        s[i * ROWS : (i + 1) * ROWS] = _unpermute_s(res.results[i]["s"], ROWS)
        mt[i * ROWS : (i + 1) * ROWS] = _unpermute_mt(res.results[i]["mt"], ROWS)

    # mt = exact per-row max (f32); accuracy = logit at the label equals it
    xlab = logits[np.arange(N), labels.astype(np.int64)]
    acc = (xlab == mt).astype(np.float32)
    conf = (np.exp(mt) / s).astype(np.float32)
    return _finish(conf, acc)
